# revision 3
# baseline (speedup 1.0000x reference)
"""ISTFT kernel for Trainium2 (8 NeuronCores, SPMD).

Math: out = trim(OLA(hann * irfft(spec)) / window_sum), FFT=2048, HOP=512.

Formulation (v3): a 2-level DIT split is applied on the HOST: output
samples n ≡ c (mod 4) form stream c, whose time series is the length-512
inverse DFT of a folded spectrum A_c (computed host-side from the input
spectrum with radix-4 butterflies + one twiddle).  On-device, each
stream is a single fp16 matmul against a constant [2048, 128] windowed
sub-DFT basis with the overlap-add fused into the K-accumulation:

    O_c[u, m] = sum_{j=0..3} X_c[u-1+j] @ D_cj      (chunk u, 128 samples)

The matmul is oriented with D stationary and the folded spectrum moving
(free dim = 512 chunks), so the PE runs fully packed: 256 matmuls/core.
Device MACs are 4x lower than the direct DFT.  The interior window-sum
(exactly 1.5 for 4x-overlap periodic hann) and 1/N are folded into D;
the first/last 512 output samples are rescaled on the host.
"""

import numpy as np

FFT = 2048
HOP = 512
B, F, NB = 4, 4000, 1025
L = (F - 1) * HOP + FFT  # 2049536 full OLA length
OUT = L - FFT            # 2047488 trimmed output length per batch
U = OUT // HOP           # 3999 output chunks per batch
NS = 512                 # sub-transform length (2048 / 4 streams)
HS = 128                 # samples per chunk per stream
SLAB = 2052              # staged padded frames per core (2048 chunks + halo)
NC_USED = 8
TINY = np.float32(np.finfo(np.float32).tiny)

_prog_cache = {}
_const_cache = {}


def _hann64(n):
    return 0.5 - 0.5 * np.cos(2.0 * np.pi * np.arange(n) / n)


def _build_constants():
    """dm [4*2048, 128] fp16 (stream-stacked windowed sub-DFT bases, OLA
    j-chunks stacked in K), edge fixups e0/e1."""
    if "dm" in _const_cache:
        return _const_cache["dm"], _const_cache["e0"], _const_cache["e1"]
    w = _hann64(FFT)
    g = 2.0 / 3.0                      # fold 1/window_sum interior (=1/1.5)
    mm = np.arange(NS)
    base = np.empty((NS, NS))          # packed-real iDFT basis [p, mm]
    kcos = np.arange(257)
    a = np.full(257, 2.0)
    a[0] = 1.0
    a[256] = 1.0
    base[:257] = a[:, None] * np.cos(2 * np.pi * np.outer(kcos, mm) / NS) / FFT
    ksin = np.arange(1, 256)
    base[257:] = -2.0 * np.sin(2 * np.pi * np.outer(ksin, mm) / NS) / FFT
    dm = np.empty((4 * 4 * NS, HS), np.float16)
    for c in range(4):
        for j in range(4):
            mloc = np.arange(HS) + HS * (3 - j)
            n = c + 4 * mloc
            dm[4 * NS * c + NS * j:4 * NS * c + NS * (j + 1)] = (
                base[:, mloc] * (w[n] * g)[None, :]
            ).astype(np.float16)

    # window_sum edge fixups for the first/last trimmed 512 samples
    w32 = w.astype(np.float32)
    wsq = np.zeros(L, np.float32)
    idx = (np.arange(F) * HOP)[:, None] + np.arange(FFT)[None, :]
    np.add.at(wsq, idx.ravel(), np.tile(w32 * w32, F))
    ws = np.where(wsq > TINY, wsq, np.float32(1.0))
    half = FFT // 2
    ws_t = ws[half:L - half]
    e0 = (np.float32(1.5) / ws_t[:HOP]).astype(np.float32)
    e1 = (np.float32(1.5) / ws_t[-HOP:]).astype(np.float32)
    _const_cache.update(dm=dm, e0=e0, e1=e1)
    return dm, e0, e1


def _build_program(reps=1):
    """Per core: for each stream c, 4 chunk-groups of 512; each group is
    16 accumulating matmuls (K = 4*NS = 4 j-shifts x 4 k'-tiles) with the
    [128,128] D tile stationary and the folded spectrum moving."""
    key = ("nc", reps)
    if key in _prog_cache:
        return _prog_cache[key]
    import concourse.bacc as bacc
    import concourse.tile as tile
    import concourse.bass as bass

    dt = bass.mybir.dt
    nc = bacc.Bacc(None, target_bir_lowering=False, debug=True)
    xt = nc.dram_tensor("xt", [4 * NS, SLAB], dt.float16, kind="ExternalInput")
    dm = nc.dram_tensor("dm", [16 * NS, HS], dt.float16, kind="ExternalInput")
    out = nc.dram_tensor("out", [4 * HS, 4 * HOP], dt.float32,
                         kind="ExternalOutput")

    with tile.TileContext(nc) as tc:
        with tc.tile_pool(name="dc", bufs=2) as dcp, \
             tc.tile_pool(name="xs", bufs=4) as xsp, \
             tc.tile_pool(name="psum", bufs=2, space="PSUM") as psump, \
             tc.tile_pool(name="osb", bufs=3) as osbp:
            for _rep in range(reps):
                for c in range(4):
                    dc = dcp.tile([128, 16, 128], dt.float16, tag="dc")
                    for t in range(16):
                        nc.sync.dma_start(
                            out=dc[:, t, :],
                            in_=dm[4 * NS * c + 128 * t:
                                   4 * NS * c + 128 * (t + 1), :],
                        )
                    for g in range(4):
                        xs = xsp.tile([128, 4, 516], dt.float16, tag="xs")
                        for tt in range(4):
                            nc.sync.dma_start(
                                out=xs[:, tt, :],
                                in_=xt[NS * c + 128 * tt:NS * c + 128 * (tt + 1),
                                       512 * g:512 * g + 516],
                            )
                        ps = psump.tile([128, HOP], dt.float32)
                        for t in range(16):
                            j, tt = t // 4, t % 4
                            nc.tensor.matmul(
                                ps[:, :],
                                dc[:, t, :],
                                xs[:, tt, j:j + 512],
                                start=(t == 0),
                                stop=(t == 15),
                            )
                        ob = osbp.tile([128, HOP], dt.float32, tag="ob")
                        nc.vector.tensor_copy(ob[:, :], ps[:, :])
                        nc.sync.dma_start(
                            out=out[HS * c:HS * (c + 1),
                                    512 * g:512 * (g + 1)],
                            in_=ob[:, :],
                        )
    nc.compile()
    _prog_cache[key] = nc
    return nc


def _fold_batch(spec_real, spec_imag):
    """[F,1025] f32 x2 -> folded packed-real streams Xp [4, F, 512] f32.

    A_c[k'] = e^{2pi i k'c/2048} * sum_r i^{rc} Shat[:, k'+512r], via
    radix-4 butterflies over the four 512-blocks of the hermitian
    extension Shat."""
    S = spec_real.astype(np.complex64)
    S.imag = spec_imag
    # blocks of Shat[k], k = k' + 512r, r = 0..3
    S0 = S[:, 0:512]
    S1 = S[:, 512:1024]
    # r=2: k = 1024+k': k' = 0 -> S[1024]; k' >= 1 -> conj(S[1024-k'])
    S2 = np.empty_like(S0)
    S2[:, 0] = S[:, 1024]
    S2[:, 1:] = np.conj(S[:, 1023:512:-1])
    S3 = np.conj(S[:, 512:0:-1])             # k=1536+k' -> conj(S[512-k'])
    E, Ed = S0 + S2, S0 - S2
    O, Od = S1 + S3, S1 - S3
    A = np.empty((4, F, NS), np.complex64)
    A[0] = E + O
    A[2] = E - O
    iOd = 1j * Od
    A[1] = Ed + iOd
    A[3] = Ed - iOd
    kp = np.arange(NS)
    for c in (1, 2, 3):
        A[c] *= np.exp((2j * np.pi * c / FFT) * kp).astype(np.complex64)
    Xp = np.empty((4, F, NS), np.float32)
    Xp[:, :, :257] = A[:, :, :257].real
    Xp[:, :, 257:] = A[:, :, 1:256].imag
    return Xp


def _run(in_maps, trace=False):
    from concourse.bass_utils import run_bass_kernel_spmd
    nc = _build_program()
    return run_bass_kernel_spmd(nc, in_maps, list(range(NC_USED)), trace=trace)


def kernel(spec_real, spec_imag, _trace=False, _ret_raw=False):
    spec_real = np.ascontiguousarray(spec_real, dtype=np.float32)
    spec_imag = np.ascontiguousarray(spec_imag, dtype=np.float32)
    dm, e0, e1 = _build_constants()

    # Stage per-core folded spectra: core (b, h) computes output chunks
    # [2000h, 2000h+2048); chunk u reads padded frames u-1+j, j=0..3, so
    # slab col 0 = global frame 2000h - 1.
    in_maps = []
    for b in range(B):
        Xp = _fold_batch(spec_real[b], spec_imag[b])     # [4, F, 512]
        for h in range(2):
            xtc = np.zeros((4 * NS, SLAB), np.float16)
            f0 = 2000 * h - 1                            # global frame of col 0
            lo = max(0, -f0)                             # first valid col
            hi = min(SLAB, F - f0)                       # end col (frame F-1)
            for c in range(4):
                xtc[NS * c:NS * (c + 1), lo:hi] = \
                    Xp[c, f0 + lo:f0 + hi].T.astype(np.float16)
            in_maps.append({"xt": xtc, "dm": dm})

    res = _run(in_maps, trace=_trace)

    chunks = np.empty((B, U, HOP), np.float32)
    for b in range(B):
        for h in range(2):
            o = res.results[2 * b + h]["out"]            # [512, 2048]
            yc = o.reshape(4, HS, 4 * HOP).transpose(2, 1, 0).reshape(-1, HOP)
            n = 2000 if h == 0 else U - 2000
            chunks[b, 2000 * h:2000 * h + n] = yc[:n]
    y = chunks.reshape(B, OUT)
    y[:, :HOP] *= e0
    y[:, -HOP:] *= e1
    if _ret_raw:
        return y, res
    return y
